# revision 47
# baseline (speedup 1.0000x reference)
"""2-layer GCN (GCNConv x2) on 8 Trainium2 NeuronCores.

Sharding: nodes (rows of x / output) sharded across 8 cores; edges
partitioned by destination core.

Math (norm separability + matmul/aggregation commutation):
    t1 = dinv*(x @ W1)                      per-node, 16 wide
    out1 = dinv*(S1 + t1) + b1,   S1[d] = sum_{e: dst=d} t1[src_e]
    t2 = dinv*relu(out1)
    out2 = (dinv*(S2 + t2)) @ W2 + b2,      S2 likewise over t2

Aggregation engine: the 16-wide tables are AllGather'd in TRANSPOSED
layout ([16 feats x 12500 nodes] per owner shard, concatenated to
[128 = (owner, feat), 12500]) and kept in SBUF.  Per-edge gathers run on
the GPSIMD Pool engine with ap_gather: each 16-partition group hosts one
owner's transposed table and gathers source rows by local id along the
free dim (no DMA descriptors per edge, no masks - padding gathers a zero
column).  Edges are bucketed per (dst-core, src-owner); each group's dst
nodes are sorted by per-group in-count so round r (the r-th edge of each
node) is a dense prefix, letting plain DVE adds accumulate rounds into a
per-group accumulator.  A final ap_gather realigns each group's
accumulator to natural node order and one 128x16 block-ones matmul per
128-node tile sums the 8 groups on the PE.
"""

import numpy as np

import concourse.bass as bass
import concourse.bacc as bacc
import concourse.mybir as mybir
import concourse.tile as tile

P = 128
NC = 8
NPC = 12500                     # nodes per core
NPAD = 12544                    # 98 * 128
TG = NPAD // P                  # 98 node tiles
HID = 16
OUT = 64
IN_CH = 128
ZR = NPC                        # zero-row index in each owner table
NE = NPC + 12                   # table num_elems (12512, %16)
R0W = NE                        # round-0 width == acc width
SLOT = 9664                     # gather ring-buffer width (%16)
CH = 512                        # phase-A / staging chunk (4 node tiles)


def _ceil16(v):
    return (v + 15) // 16 * 16


def _host_prep(x, edge_index):
    N, _ = x.shape
    src = np.asarray(edge_index[0], np.int64)
    dst = np.asarray(edge_index[1], np.int64)
    deg = np.bincount(dst, minlength=N)
    dinv = (1.0 / np.sqrt(deg + 1.0)).astype(np.float32)
    x_pre = np.asarray(x, np.float32) * dinv[:, None]

    cores = []
    for m in range(NC):
        sel = (dst >= m * NPC) & (dst < (m + 1) * NPC)
        ed = dst[sel] - m * NPC
        es = src[sel]
        own = es // NPC
        sl = es % NPC
        groups = []
        for o in range(NC):
            g = own == o
            d_o = ed[g]
            s_o = sl[g]
            c = np.bincount(d_o, minlength=NPC)
            order_e = np.argsort(d_o, kind="stable")
            csr = s_o[order_e]
            indptr = np.zeros(NPC + 1, np.int64)
            indptr[1:] = np.cumsum(c)
            order = np.argsort(-c, kind="stable")
            rank = np.empty(NPC, np.int64)
            rank[order] = np.arange(NPC)
            groups.append(dict(c=c, csr=csr, indptr=indptr,
                               order=order, rank=rank))
        cores.append(groups)

    # global (all cores x groups) round widths so the SPMD program is uniform
    rmax = int(max(g["c"].max() for gs in cores for g in gs))
    n_r = [R0W]
    for r in range(1, rmax):
        w = max(int((g["c"] > r).sum()) for gs in cores for g in gs)
        if w == 0:
            break
        n_r.append(w)
    L = sum(n_r)
    Lpad = _ceil16(L)

    # call plan: call 0 = round 0 into acc; rest chunked into the two
    # ring slots (width <= SLOT, %16)
    rem = Lpad - R0W
    k = -(-rem // SLOT)
    calls = [(0, R0W)]
    lo = R0W
    for i in range(k):
        w = min(_ceil16(-(-rem // k)), Lpad - lo) if i < k - 1 else Lpad - lo
        calls.append((lo, lo + w))
        lo += w
    assert lo == Lpad and all(b - a <= max(SLOT, R0W) for a, b in calls)

    # DVE add schedule: round r>=1 stream segment [s_r, s_r+n_r) ->
    # per intersecting call: (call idx, local col, width, acc offset)
    adds = []
    s = R0W
    for r in range(1, len(n_r)):
        seg = (s, s + n_r[r])
        for ci, (a, b) in enumerate(calls):
            il, ih = max(seg[0], a), min(seg[1], b)
            if il < ih:
                adds.append((ci, il - a, ih - il, il - seg[0]))
        s += n_r[r]

    per_core = []
    for m in range(NC):
        idx_arr = np.zeros((P, Lpad // 16), np.int16)
        ridx = np.zeros((P, NPAD // 16), np.int16)
        for o in range(NC):
            g = cores[m][o]
            stream = np.full(Lpad, ZR, np.int64)
            pos = 0
            for r, w in enumerate(n_r):
                take = min(w, NPC)
                nodes = g["order"][:take]
                valid = g["c"][nodes] > r
                if len(g["csr"]):
                    bi = np.minimum(g["indptr"][nodes] + r, len(g["csr"]) - 1)
                    vals = np.where(valid, g["csr"][bi], ZR)
                else:
                    vals = np.full(take, ZR, np.int64)
                stream[pos:pos + take] = vals
                pos += w
            idx_arr[16 * o:16 * (o + 1), :] = \
                stream.reshape(-1, 16).T.astype(np.int16)
            rs = np.full(NPAD, ZR, np.int64)
            rs[:NPC] = g["rank"]
            ridx[16 * o:16 * (o + 1), :] = rs.reshape(-1, 16).T.astype(np.int16)

        import ml_dtypes
        xpt = np.zeros((P, NPAD), ml_dtypes.bfloat16)
        xpt[:, :NPC] = x_pre[m * NPC:(m + 1) * NPC].T.astype(ml_dtypes.bfloat16)
        dn = np.zeros(NPAD, np.float32)
        dn[:NPC] = dinv[m * NPC:(m + 1) * NPC]
        per_core.append(dict(
            xpt=xpt, idxs=idx_arr, ridx=ridx,
            dinv_nm=dn.reshape(TG, P).T.copy()))

    ones_sel = np.zeros((P, HID), np.float32)
    ones_sel[np.arange(P), np.arange(P) % HID] = 1.0
    meta = dict(Lpad=Lpad, calls=calls, adds=adds, n_r=n_r, ones=ones_sel)
    return per_core, meta


def _build_nc(meta):
    dt = mybir.dt
    f32, i16 = dt.float32, dt.int16
    Lpad, calls, adds = meta["Lpad"], meta["calls"], meta["adds"]
    mult, add = mybir.AluOpType.mult, mybir.AluOpType.add
    groups = [list(range(NC))]

    bf16 = dt.bfloat16
    nc = bacc.Bacc(num_devices=NC, num_swdge_queues=4)
    xpt_d = nc.declare_dram_parameter("xpt", [P, NPAD], bf16, isOutput=False)
    idx_d = nc.declare_dram_parameter("idxs", [P, Lpad // 16], i16, isOutput=False)
    ridx_d = nc.declare_dram_parameter("ridx", [P, NPAD // 16], i16, isOutput=False)
    dinv_d = nc.declare_dram_parameter("dinv_nm", [P, TG], f32, isOutput=False)
    w1_d = nc.declare_dram_parameter("w1", [IN_CH, HID], bf16, isOutput=False)
    w2_d = nc.declare_dram_parameter("w2", [HID, OUT], bf16, isOutput=False)
    b1_d = nc.declare_dram_parameter("b1r", [P, HID], f32, isOutput=False)
    b2_d = nc.declare_dram_parameter("b2r", [P, OUT], f32, isOutput=False)
    ones_d = nc.declare_dram_parameter("ones", [P, HID], f32, isOutput=False)
    ident_d = nc.declare_dram_parameter("ident", [P, P], bf16, isOutput=False)
    out_d = nc.declare_dram_parameter("out_shard", [NPAD, OUT], f32, isOutput=True)

    t1t = nc.dram_tensor("t1t", [HID, NPC], bf16)
    t2t = nc.dram_tensor("t2t", [HID, NPC], bf16)
    tab1 = nc.dram_tensor("tab1", [P, NPC], bf16)
    tab2 = nc.dram_tensor("tab2", [P, NPC], bf16)

    with tile.TileContext(nc) as tc:
        with tc.tile_pool(name="big", bufs=1) as bp, \
             tc.tile_pool(name="work", bufs=2) as wp, \
             tc.tile_pool(name="psum", bufs=2, space="PSUM") as pp:

            ident = bp.tile([P, P], bf16)
            nc.sync.dma_start(out=ident[:], in_=ident_d[:])
            w1_s = bp.tile([IN_CH, HID], bf16)
            nc.sync.dma_start(out=w1_s[:], in_=w1_d[:])
            w2_s = bp.tile([HID, OUT], bf16)
            nc.sync.dma_start(out=w2_s[:], in_=w2_d[:])
            b1_s = bp.tile([P, HID], f32)
            nc.sync.dma_start(out=b1_s[:], in_=b1_d[:])
            b2_s = bp.tile([P, OUT], f32)
            nc.sync.dma_start(out=b2_s[:], in_=b2_d[:])
            ones_s = bp.tile([P, HID], f32)
            nc.sync.dma_start(out=ones_s[:], in_=ones_d[:])
            dinv_s = bp.tile([P, TG], f32)
            nc.sync.dma_start(out=dinv_s[:], in_=dinv_d[:])
            table = bp.tile([P, NPAD], f32)       # gather table / acc_nat
            acc = bp.tile([P, NE], f32)           # per-group accumulator
            bufs = [bp.tile([P, SLOT], f32, tag=f"slot{i}", name=f"slotbuf{i}")
                    for i in range(2)]
            t1nm = bp.tile([P, TG * HID], bf16)   # t1 node-major (also z)
            t2nm = bp.tile([P, TG * HID], bf16)
            nc.vector.memset(table[:, NPC:NPAD], 0.0)
            IXW = max(SLOT, R0W, NPAD) // 16

            def load_idx(a, b):
                ix = wp.tile([P, IXW], i16, tag="ix")
                nc.sync.dma_start(out=ix[:, 0:(b - a) // 16],
                                  in_=idx_d[:, a // 16:b // 16])
                return ix

            # ---------- phase A: t1T = (dinv*x @ W1).T streamed to DRAM,
            # plus node-major copy t1nm
            for c0 in range(0, NPAD, CH):
                w = min(CH, NPAD - c0)
                xt = wp.tile([P, CH], bf16, tag="xt", bufs=3)
                nc.gpsimd.dma_start(out=xt[:, 0:w], in_=xpt_d[:, c0:c0 + w])
                pt = pp.tile([HID, CH], f32, tag="ch16")
                nc.tensor.matmul(pt[:, 0:w], lhsT=w1_s[:], rhs=xt[:, 0:w],
                                 start=True, stop=True)
                st = wp.tile([HID, 2 * CH], bf16, tag="st16", bufs=3)
                nc.scalar.copy(out=st[:, 0:w], in_=pt[:, 0:w])
                wd = min(w, NPC - c0)
                if wd > 0:
                    nc.sync.dma_start(out=t1t[:, c0:c0 + wd],
                                      in_=st[:, 0:wd])
                pn = pp.tile([P, 8 * OUT], f32, tag="nmx")
                for k in range(w // P):
                    nc.tensor.matmul(pn[:, k * HID:(k + 1) * HID],
                                     lhsT=xt[:, k * P:(k + 1) * P], rhs=w1_s[:],
                                     start=True, stop=True)
                t0 = c0 // P
                nc.scalar.copy(
                    out=t1nm[:, t0 * HID:(t0 + w // P) * HID],
                    in_=pn[:, 0:w // P * HID])

            def load_table(tab):
                # stage bf16 into acc (dead here), convert to f32 gather
                # table; chunked so DMA overlaps DVE/ACT converts
                stage = acc[:, 0:NPC // 2].bitcast(bf16)
                ck = NPC // 4
                for i in range(4):
                    lo, hi = i * ck, (i + 1) * ck
                    nc.sync.dma_start(out=stage[:, lo:hi], in_=tab[:, lo:hi])
                    if i % 2 == 0:
                        nc.vector.tensor_copy(out=table[:, lo:hi],
                                              in_=stage[:, lo:hi])
                    else:
                        nc.scalar.copy(out=table[:, lo:hi],
                                       in_=stage[:, lo:hi])

            nc.gpsimd.collective_compute(
                "AllGather", mybir.AluOpType.bypass, replica_groups=groups,
                ins=[t1t[:].rearrange("p n -> (p n)")],
                outs=[tab1[:].rearrange("p n -> (p n)")])
            load_table(tab1)

            def aggregate(layer):
                # gather rounds; round 0 lands in acc, rest ring-buffered
                for ci, (a, b) in enumerate(calls):
                    w = b - a
                    dst = acc if ci == 0 else bufs[(ci - 1) % 2]
                    ix = load_idx(a, b)
                    nc.gpsimd.ap_gather(
                        out_ap=dst[:, 0:w].rearrange("p (n d) -> p n d", d=1),
                        in_ap=table[:, 0:NE].rearrange("p (n d) -> p n d", d=1),
                        idxs_ap=ix[:, 0:w // 16],
                        channels=P, num_elems=NE, d=1, num_idxs=w)
                    for (aci, lo, wdt, off) in adds:
                        if aci != ci:
                            continue
                        src = acc if ci == 0 else bufs[(ci - 1) % 2]
                        nc.vector.tensor_add(acc[:, off:off + wdt],
                                             acc[:, off:off + wdt],
                                             src[:, lo:lo + wdt])
                # realign each group's acc to natural node order (into table,
                # which is dead until the next layer's table DMA)
                rx = wp.tile([P, IXW], i16, tag="ix")
                nc.sync.dma_start(out=rx[:, 0:NPAD // 16], in_=ridx_d[:])
                nc.gpsimd.ap_gather(
                    out_ap=table[:].rearrange("p (n d) -> p n d", d=1),
                    in_ap=acc[:].rearrange("p (n d) -> p n d", d=1),
                    idxs_ap=rx[:, 0:NPAD // 16],
                    channels=P, num_elems=NE, d=1, num_idxs=NPAD)

            # cross-group sum: agg[(t p), f] = sum_o table[(o f), t*128+p]
            def cross_group(t0, nt):
                pa = pp.tile([P, 32 * HID], f32, tag="agg")
                for j in range(nt):
                    t = t0 + j
                    nc.tensor.matmul(pa[:, j * HID:(j + 1) * HID],
                                     lhsT=table[:, t * P:(t + 1) * P],
                                     rhs=ones_s[:], start=True, stop=True)
                return pa

            # ---------------- layer 1
            aggregate(1)
            for t0 in range(0, TG, 32):
                nt = min(32, TG - t0)
                pa = cross_group(t0, nt)
                w = nt * HID
                sl = slice(t0 * HID, t0 * HID + w)
                a3 = pa[:, 0:w].rearrange("p (t f) -> p t f", f=HID)
                d3 = dinv_s[:, t0:t0 + nt, None].to_broadcast([P, nt, HID])
                o3 = t2nm[:, sl].rearrange("p (t f) -> p t f", f=HID)
                nc.vector.tensor_add(pa[:, 0:w], pa[:, 0:w], t1nm[:, sl])
                nc.vector.tensor_tensor(out=a3, in0=a3, in1=d3, op=mult)
                nc.vector.tensor_tensor(
                    out=a3, in0=a3,
                    in1=b1_s[:, None, :].to_broadcast([P, nt, HID]), op=add)
                nc.vector.tensor_relu(out=pa[:, 0:w], in_=pa[:, 0:w])
                nc.vector.tensor_tensor(out=o3, in0=a3, in1=d3, op=mult)

            # t2T -> DRAM -> AllGather -> table
            for c0 in range(0, NPAD, 2 * CH):
                w = min(2 * CH, NPAD - c0)
                pt = pp.tile([HID, 2 * CH], bf16, tag="tp16")
                for k in range(w // P):
                    t = c0 // P + k
                    nc.tensor.transpose(pt[:, k * P:(k + 1) * P],
                                        t2nm[:, t * HID:(t + 1) * HID],
                                        ident[:])
                st = wp.tile([HID, 2 * CH], bf16, tag="st16", bufs=3)
                nc.scalar.copy(out=st[:, 0:w], in_=pt[:, 0:w])
                wd = min(w, NPC - c0)
                if wd > 0:
                    nc.sync.dma_start(out=t2t[:, c0:c0 + wd], in_=st[:, 0:wd])
            nc.gpsimd.collective_compute(
                "AllGather", mybir.AluOpType.bypass, replica_groups=groups,
                ins=[t2t[:].rearrange("p n -> (p n)")],
                outs=[tab2[:].rearrange("p n -> (p n)")])
            load_table(tab2)

            # ---------------- layer 2
            aggregate(2)
            z = t1nm  # dead, reuse as z = dinv*(agg2 + t2nm)
            for t0 in range(0, TG, 32):
                nt = min(32, TG - t0)
                pa = cross_group(t0, nt)
                w = nt * HID
                sl = slice(t0 * HID, t0 * HID + w)
                a3 = pa[:, 0:w].rearrange("p (t f) -> p t f", f=HID)
                d3 = dinv_s[:, t0:t0 + nt, None].to_broadcast([P, nt, HID])
                z3 = z[:, sl].rearrange("p (t f) -> p t f", f=HID)
                nc.vector.tensor_add(pa[:, 0:w], pa[:, 0:w], t2nm[:, sl])
                nc.vector.tensor_tensor(out=z3, in0=a3, in1=d3, op=mult)

            # out2 = z @ W2 + b2, node-major, batched 8 tiles per PSUM bank
            for t0 in range(0, TG, 8):
                nt = min(8, TG - t0)
                pz = pp.tile([HID, 2 * CH], bf16, tag="tp16")
                for k in range(nt):
                    t = t0 + k
                    nc.tensor.transpose(pz[:, k * P:(k + 1) * P],
                                        z[:, t * HID:(t + 1) * HID], ident[:])
                zs = wp.tile([HID, 2 * CH], bf16, tag="st16", bufs=3)
                nc.scalar.copy(out=zs[:, 0:nt * P], in_=pz[:, 0:nt * P])
                po = pp.tile([P, 8 * OUT], f32, tag="nmx")
                for k in range(nt):
                    nc.tensor.matmul(po[:, k * OUT:(k + 1) * OUT],
                                     lhsT=zs[:, k * P:(k + 1) * P], rhs=w2_s[:],
                                     start=True, stop=True)
                w = nt * OUT
                os_ = wp.tile([P, 8 * OUT], f32, tag="os", bufs=3)
                nc.vector.tensor_tensor(
                    out=os_[:, 0:w].rearrange("p (t f) -> p t f", f=OUT),
                    in0=po[:, 0:w].rearrange("p (t f) -> p t f", f=OUT),
                    in1=b2_s[:, None, :].to_broadcast([P, nt, OUT]), op=add)
                nc.sync.dma_start(
                    out=out_d[t0 * P:(t0 + nt) * P, :].rearrange(
                        "(t p) f -> p t f", p=P),
                    in_=os_[:, 0:w].rearrange("p (t f) -> p t f", f=OUT))

    nc.finalize()
    return nc


def build_full(x, edge_index, W1, b1, W2, b2, n_cores=8):
    per_core, meta = _host_prep(np.asarray(x), np.asarray(edge_index))
    nc = _build_nc(meta)
    import ml_dtypes
    bf = ml_dtypes.bfloat16
    common = dict(
        w1=np.asarray(W1, np.float32).astype(bf),
        w2=np.asarray(W2, np.float32).astype(bf),
        b1r=np.broadcast_to(np.asarray(b1, np.float32), (P, HID)).copy(),
        b2r=np.broadcast_to(np.asarray(b2, np.float32), (P, OUT)).copy(),
        ones=meta["ones"], ident=np.eye(P, dtype=bf))
    in_maps = [dict(**pc, **common) for pc in per_core]
    return nc, in_maps, meta


def unpermute_output(results, meta, n_cores=8):
    return np.concatenate(
        [results[m]["out_shard"][:NPC] for m in range(n_cores)])


def _run(x, edge_index, W1, b1, W2, b2, n_cores=8):
    nc, in_maps, meta = build_full(x, edge_index, W1, b1, W2, b2, n_cores)
    r = PjrtRunner(nc, n_cores)
    results = r.run(in_maps)
    return unpermute_output(results, meta, n_cores), r


def kernel(**inputs) -> np.ndarray:
    return _run(inputs["x"], inputs["edge_index"], inputs["W1"], inputs["b1"],
                inputs["W2"], inputs["b2"], n_cores=8)[0]


class PjrtRunner:
    """run_bass_via_pjrt with a persistent jitted executable, so repeated
    executions (for wall-clock timing) skip retracing/recompiling."""

    def __init__(self, nc, n_cores):
        import jax
        from jax.experimental.shard_map import shard_map
        from jax.sharding import Mesh, PartitionSpec
        from concourse import bass2jax, mybir as mb

        bass2jax.install_neuronx_cc_hook()
        self.nc = nc
        self.n_cores = n_cores
        partition_name = (nc.partition_id_tensor.name
                          if nc.partition_id_tensor else None)
        in_names, out_names, out_avals, zero_outs = [], [], [], []
        for alloc in nc.m.functions[0].allocations:
            if not isinstance(alloc, mb.MemoryLocationSet):
                continue
            name = alloc.memorylocations[0].name
            if alloc.kind == "ExternalInput":
                if name != partition_name:
                    in_names.append(name)
            elif alloc.kind == "ExternalOutput":
                shape = tuple(alloc.tensor_shape)
                dtype = mb.dt.np(alloc.dtype)
                out_names.append(name)
                out_avals.append(jax.core.ShapedArray(shape, dtype))
                zero_outs.append(np.zeros(shape, dtype))
        self.in_names, self.out_names = in_names, out_names
        self.out_avals, self.zero_outs = out_avals, zero_outs
        n_params, n_outs = len(in_names), len(out_avals)
        self.n_params = n_params
        all_names = in_names + out_names
        if partition_name is not None:
            all_names.append(partition_name)

        def _body(*args):
            operands = list(args)
            if partition_name is not None:
                operands.append(bass2jax.partition_id_tensor())
            outs = bass2jax._bass_exec_p.bind(
                *operands, out_avals=tuple(out_avals),
                in_names=tuple(all_names), out_names=tuple(out_names),
                lowering_input_output_aliases=(),
                sim_require_finite=True, sim_require_nnan=True, nc=nc)
            return tuple(outs)

        devices = jax.devices()[:n_cores]
        self.mesh = Mesh(np.asarray(devices), ("core",))
        donate = tuple(range(n_params, n_params + n_outs))
        self.sharded = jax.jit(
            shard_map(_body, mesh=self.mesh,
                      in_specs=(PartitionSpec("core"),) * (n_params + n_outs),
                      out_specs=(PartitionSpec("core"),) * n_outs,
                      check_rep=False),
            donate_argnums=donate, keep_unused=True)
        self.jax = jax
        self._dev_in = None

    def put_inputs(self, in_maps):
        concat = [np.concatenate([np.asarray(in_maps[c][n])
                                  for c in range(self.n_cores)], axis=0)
                  for n in self.in_names]
        self._dev_in = [self.jax.device_put(a) for a in concat]

    def _fresh_zeros(self):
        return [np.zeros((self.n_cores * z.shape[0], *z.shape[1:]), z.dtype)
                for z in self.zero_outs]

    def execute(self):
        outs = self.sharded(*self._dev_in, *self._fresh_zeros())
        self.jax.block_until_ready(outs)
        return outs

    def run(self, in_maps):
        self.put_inputs(in_maps)
        outs = self.execute()
        return [
            {n: np.asarray(outs[i]).reshape(self.n_cores,
                                            *self.out_avals[i].shape)[c]
             for i, n in enumerate(self.out_names)}
            for c in range(self.n_cores)
        ]

    def bench(self, iters=5):
        import time
        zeros = [self._fresh_zeros() for _ in range(iters)]
        times = []
        for z in zeros:
            t0 = time.perf_counter()
            outs = self.sharded(*self._dev_in, *z)
            self.jax.block_until_ready(outs)
            times.append(time.perf_counter() - t0)
        return times
